# revision 1
# baseline (speedup 1.0000x reference)
"""Multi-head attention (B=8, S=1024, D=768, H=12) on 8 TRN2 NeuronCores.

Sharding: pure batch parallelism — one batch element per core, weights
replicated. No collectives needed.

Per-core pipeline (tokens T=1024, D=768, H=12 heads of HD=64):
  1. Load x [T, D], PE-transpose to xT [D, T]; load W_qkv staged through
     SBUF and round fp32 -> tf32 (fp32r) so the PE runs at 1 cycle/row.
  2. QKV projections as fp32r matmuls:
       V [T, 768] = xT-chunks^T @ W_qkv[:, 1536:]       (stored bf16)
       Q^T, K^T [768, T] = W_qkv[:, :1536]-chunks^T @ xT (kept fp32r)
     QK chunk pairs are software-pipelined with the attention heads that
     consume them, so PE-heavy projection overlaps DVE/ACT-heavy softmax.
  3. Per head h, per query chunk qi (128 queries, causal k <= (qi+1)*128):
       s = Q_h K_h^T (fp32r), diagonal block masked with -1e10
       m = rowmax(s) on DVE; w = exp(8*s - 8*m) -> bf16 on ACT, with the
       row sums accumulated by the same ACT instruction
       w blocks PE-transposed to wT [k, q] (bf16)
       o = w @ V_h accumulated over k chunks (bf16 matmul, N=64)
       attn[:, h*64:] = o * (1/rowsum)  (stored bf16)
  4. In the last head's sweep each finished token chunk is immediately
     PE-transposed to attnT and projected: y = attn @ W_proj + b (bf16
     matmul, fp32 accumulate), then DMA'd out.

Measured vs the fp32 jax reference: rel err ~3.2e-3 on hardware (scores
and softmax stats in fp32/tf32; only w/V/attn/W_proj are bf16).
"""

import numpy as np

import concourse.bass as bass
import concourse.mybir as mybir
import concourse.tile as tile
from concourse import bacc
from concourse.bass_utils import run_bass_kernel_spmd
from concourse.masks import make_causal_mask, make_identity

B, S, D = 8, 1024, 768
H, HD = 12, 64
HV = 65  # V block width per head: 64 value cols + a ones col whose AV
         # matmul output column is the softmax denominator
NT = S // 128   # 8 token chunks
ND = D // 128   # 6 d chunks
F32 = mybir.dt.float32
F32R = mybir.dt.float32r
BF16 = mybir.dt.bfloat16

N_CORES = 8


def bank_chunks(size):
    """Split [0, size) into matmul-N chunks that each sit in one PSUM bank
    (fp32 bank = 512 elems) and are >=256 where possible (fp32r full rate)."""
    out = []
    start = 0
    while start < size:
        end = min(start + 512, size, (start // 512 + 1) * 512)
        out.append((start, end))
        start = end
    return out


def build_mha(nc):
    x_d = nc.dram_tensor("x", [S, D], F32, kind="ExternalInput")
    wqkv_d = nc.dram_tensor("W_qkv", [D, 3 * D], F32, kind="ExternalInput")
    wproj_d = nc.dram_tensor("W_proj", [D, D], F32, kind="ExternalInput")
    bproj_d = nc.dram_tensor("b_proj", [1, D], F32, kind="ExternalInput")
    out_d = nc.dram_tensor("out", [S, D], F32, kind="ExternalOutput")

    with tile.TileContext(nc) as tc:
        with (
            tc.tile_pool(name="persist", bufs=1) as pp,
            tc.tile_pool(name="psum", bufs=1, space="PSUM") as psum,
        ):
            def ptile():
                return psum.tile([128, 1024], F32, name="p1", tag="pbig", bufs=3)

            def ptiny():
                return psum.tile([128, 512], F32, name="pt1", tag="ptiny", bufs=2)

            # ---- constants ----
            ident_f32 = pp.tile([128, 128], F32, name="ident_f32", tag="ident_f32")
            make_identity(nc, ident_f32[:])
            ident_bf16 = pp.tile([128, 128], BF16, name="ident_bf16", tag="ident_bf16")
            nc.vector.tensor_copy(ident_bf16[:], ident_f32[:])
            # bigmask: [0, S) columns are zero, [S, S+128) hold the causal
            # block mask; slicing aligns the mask with the diagonal block
            bigmask = pp.tile([128, S + 128], F32, name="bigmask", tag="bigmask")
            nc.gpsimd.memset(bigmask[:], 0.0)
            make_causal_mask(nc, bigmask[:, S:S + 128], mask_val=-1e10)

            # b_proj broadcast to 128 partitions via K=1 outer product
            b_row = pp.tile([1, D], F32, name="b_row", tag="b_row")
            nc.sync.dma_start(b_row[:], bproj_d[:])
            ones_col = pp.tile([1, 128], F32, name="ones_col", tag="ones_col")
            nc.vector.memset(ones_col[:], 1.0)
            b_bcast = pp.tile([128, D], F32, name="b_bcast", tag="b_bcast")
            pb = ptile()
            for c0, c1 in bank_chunks(D):
                nc.tensor.matmul(
                    pb[:, c0:c1], ones_col[:], b_row[:, c0:c1],
                    start=True, stop=True,
                )
            nc.vector.tensor_copy(b_bcast[:], pb[:, :D])

            # ---- persistent activations ----
            qkT = [pp.tile([128, S], F32R, name=f"qkT{m}", tag=f"qkT{m}") for m in range(12)]
            v_sb = [pp.tile([128, H * HV], BF16, name=f"v{qi}", tag=f"v{qi}") for qi in range(NT)]
            attn = [pp.tile([128, D], BF16, name=f"attn{qi}", tag=f"attn{qi}") for qi in range(NT)]
            attnT = [pp.tile([128, S], BF16, name=f"attnT{di}", tag=f"attnT{di}") for di in range(ND)]
            wp = [pp.tile([128, D], BF16, name=f"wp{di}", tag=f"wp{di}") for di in range(ND)]
            for di in range(ND):
                # SWDGE cast fp32 -> bf16 during load
                nc.gpsimd.dma_start(
                    wp[di][:], wproj_d[di * 128:(di + 1) * 128, :]
                )

            with (
                tc.tile_pool(name="ph2", bufs=1) as p2,
                tc.tile_pool(name="xpool", bufs=2) as xp,
                tc.tile_pool(name="ph3", bufs=4) as p3,
                tc.tile_pool(name="ph3s", bufs=3) as p3s,
                tc.tile_pool(name="ypool", bufs=1) as yp,
            ):
                # ---- x load + transpose ----
                xT = [p2.tile([128, S], F32R, name=f"xT{di}", tag=f"xT{di}") for di in range(ND)]
                for qi in range(NT):
                    x_t = xp.tile([128, D], F32, name="x_t", tag="x_t")
                    nc.sync.dma_start(x_t[:], x_d[qi * 128:(qi + 1) * 128, :])
                    for di in range(ND):
                        pt = ptiny()
                        nc.tensor.transpose(
                            pt[:, :128], x_t[:, di * 128:(di + 1) * 128],
                            ident_f32[:]
                        )
                        nc.vector.tensor_copy(
                            xT[di][:, qi * 128:(qi + 1) * 128], pt[:, :128]
                        )

                # ---- W_qkv load + tf32 rounding, V columns first so the V
                # projection (and then attention) can start early ----
                wq = [p2.tile([128, 3 * D], F32R, name=f"wq{di}", tag=f"wq{di}") for di in range(ND)]
                n_stage = 0
                for part in (2, 0, 1):
                    for di in range(ND):
                        tag = "x_t" if n_stage % 2 == 0 else "y_t"
                        pool = xp if n_stage % 2 == 0 else yp
                        wq_stage = pool.tile([128, D], F32, name="wq_stage", tag=tag)
                        eng = nc.sync if n_stage % 2 == 0 else nc.scalar
                        n_stage += 1
                        eng.dma_start(
                            wq_stage[:],
                            wqkv_d[di * 128:(di + 1) * 128,
                                   part * D:(part + 1) * D],
                        )
                        # rounds fp32 -> tf32 (fp32r) for full-rate PE use
                        nc.vector.tensor_copy(
                            wq[di][:, part * D:(part + 1) * D], wq_stage[:]
                        )

                # ---- V in [token, dv] layout, with ones column per head ----
                for qi in range(NT):
                    pv = ptile()
                    for c0, c1 in bank_chunks(D):
                        for di in range(ND):
                            nc.tensor.matmul(
                                pv[:, c0:c1],
                                xT[di][:, qi * 128:(qi + 1) * 128],
                                wq[di][:, 2 * D + c0:2 * D + c1],
                                start=(di == 0), stop=(di == ND - 1),
                            )
                    nc.gpsimd.memset(
                        v_sb[qi][:].rearrange("p (h v) -> p h v", v=HV)[:, :, HD:], 1.0
                    )
                    nc.vector.tensor_copy(
                        v_sb[qi][:].rearrange("p (h v) -> p h v", v=HV)[:, :, :HD],
                        pv[:, :D].rearrange("p (h v) -> p h v", v=HD),
                    )

                def qk_chunk(m):
                    pqk = ptile()
                    for c0, c1 in bank_chunks(S):
                        for di in range(ND):
                            nc.tensor.matmul(
                                pqk[:, c0:c1],
                                wq[di][:, m * 128:(m + 1) * 128],
                                xT[di][:, c0:c1],
                                start=(di == 0), stop=(di == ND - 1),
                            )
                    if m < 6:
                        # pre-scale Q by 8: scores then come out as 8*s and
                        # the softmax needs no separate x8 pass
                        nc.scalar.mul(qkT[m][:], pqk[:], 8.0)
                    else:
                        nc.vector.tensor_copy(qkT[m][:], pqk[:])

                def attention_head(h):
                    qoff = (h % 2) * 64
                    Qt = qkT[h // 2]
                    Kt = qkT[6 + h // 2]
                    for qi in range(NT):
                        ks = (qi + 1) * 128
                        lhs = Qt[qoff:qoff + 64, qi * 128:(qi + 1) * 128]
                        ps = ptile()
                        for c0, c1 in bank_chunks(ks):
                            nc.tensor.matmul(
                                ps[:, c0:c1],
                                lhs,
                                Kt[qoff:qoff + 64, c0:c1],
                                start=True, stop=True,
                            )
                        # causal mask on the diagonal block
                        nc.vector.tensor_tensor(
                            out=ps[:, qi * 128:ks],
                            in0=ps[:, qi * 128:ks],
                            in1=bigmask[:, S:S + 128],
                            op=mybir.AluOpType.add,
                        )
                        neg8m = p3s.tile([128, 1], F32, name="neg8m", tag="neg8m", bufs=6)
                        nc.vector.reduce_max(
                            out=neg8m[:], in_=ps[:, :ks],
                            axis=mybir.AxisListType.X, negate=True,
                        )
                        w_t = p3s.tile([128, S], BF16, name="w_t", tag="w_t")
                        nc.scalar.activation(
                            w_t[:, :ks], ps[:, :ks],
                            mybir.ActivationFunctionType.Exp,
                            bias=neg8m[:], scale=1.0,
                        )

                        # w[q, ki*128:] -> wT[:, ki, q] via PE transpose;
                        # all blocks stage through one 1-bank psum tile and
                        # evacuate in a single ACT copy
                        wT = p3.tile([128, NT, 128], BF16, name="wT", tag="wT")
                        pt8 = ptiny().bitcast(BF16).rearrange(
                            "p (b q) -> p b q", q=128
                        )
                        for ki in range(qi + 1):
                            nc.tensor.transpose(
                                pt8[:, ki, :],
                                w_t[:, ki * 128:(ki + 1) * 128],
                                ident_bf16[:],
                            )
                        nc.scalar.copy(
                            wT[:, :qi + 1, :], pt8[:, :qi + 1, :]
                        )

                        # o = w @ [V_h | 1]; last column = softmax denominator
                        po = ptiny()
                        for ki in range(qi + 1):
                            nc.tensor.matmul(
                                po[:, :HV],
                                wT[:, ki, :],
                                v_sb[ki][:, h * HV:(h + 1) * HV],
                                start=(ki == 0), stop=(ki == qi),
                            )
                        recip = p3s.tile([128, 1], F32, name="recip", tag="recip", bufs=6)
                        nc.vector.reciprocal(recip[:], po[:, HD:HV])
                        nc.vector.tensor_scalar_mul(
                            attn[qi][:, h * HD:(h + 1) * HD],
                            po[:, :HD],
                            recip[:],
                        )

                        if h == H - 1:
                            # all heads done for token chunk qi: project now
                            for di0 in range(0, ND, 4):
                                nb = min(4, ND - di0)
                                pat = ptiny().bitcast(BF16).rearrange(
                                    "p (b q) -> p b q", q=128
                                )
                                for j in range(nb):
                                    di = di0 + j
                                    nc.tensor.transpose(
                                        pat[:, j, :],
                                        attn[qi][:, di * 128:(di + 1) * 128],
                                        ident_bf16[:],
                                    )
                                for j in range(nb):
                                    di = di0 + j
                                    nc.scalar.copy(
                                        attnT[di][:, qi * 128:(qi + 1) * 128],
                                        pat[:, j, :],
                                    )
                            y_t = yp.tile([128, D], F32, name="y_t", tag="y_t")
                            py = ptile()
                            for c0, c1 in bank_chunks(D):
                                for di in range(ND):
                                    nc.tensor.matmul(
                                        py[:, c0:c1],
                                        attnT[di][:, qi * 128:(qi + 1) * 128],
                                        wp[di][:, c0:c1],
                                        start=(di == 0), stop=(di == ND - 1),
                                    )
                            nc.vector.tensor_tensor(
                                out=y_t[:], in0=py[:, :D], in1=b_bcast[:],
                                op=mybir.AluOpType.add,
                            )
                            nc.sync.dma_start(
                                out_d[qi * 128:(qi + 1) * 128, :], y_t[:]
                            )

                # software pipeline: each QK chunk pair immediately feeds the
                # two heads that consume it, so PE-heavy QK overlaps the
                # DVE/ACT-heavy softmax of previous heads
                for r in range(6):
                    qk_chunk(r)
                    qk_chunk(6 + r)
                    attention_head(2 * r)
                    attention_head(2 * r + 1)

    nc.compile()
    return nc


_NC_CACHE = None


def _get_nc():
    global _NC_CACHE
    if _NC_CACHE is None:
        nc = bacc.Bacc(
            "TRN2",
            target_bir_lowering=False,
            debug=False,
            num_devices=N_CORES,
        )
        build_mha(nc)
        _NC_CACHE = nc
    return _NC_CACHE


def kernel(x, W_qkv, W_proj, b_proj):
    nc = _get_nc()
    x = np.ascontiguousarray(np.asarray(x, dtype=np.float32))
    W_qkv = np.ascontiguousarray(np.asarray(W_qkv, dtype=np.float32))
    W_proj = np.ascontiguousarray(np.asarray(W_proj, dtype=np.float32))
    b_proj = np.ascontiguousarray(
        np.asarray(b_proj, dtype=np.float32).reshape(1, D)
    )
    in_maps = [
        {"x": x[b], "W_qkv": W_qkv, "W_proj": W_proj, "b_proj": b_proj}
        for b in range(N_CORES)
    ]
    res = run_bass_kernel_spmd(nc, in_maps, core_ids=list(range(N_CORES)))
    return np.stack([res.results[b]["out"] for b in range(N_CORES)], axis=0)



# revision 6
# speedup vs baseline: 1.1886x; 1.1886x over previous
"""Multi-head attention (B=8, S=1024, D=768, H=12) on 8 TRN2 NeuronCores.

Sharding: pure batch parallelism - one batch element per core, weights
replicated. No collectives.

Per-core structure (tokens S=1024, D=768, H=12 heads of HD=64):
  - x is DMA'd first (the DMA fabric is the prologue's serial resource),
    PE-transposed to xT [d, tok] via a bf16 identity (1 cyc/row).
  - W_qkv lands as 18 column-slice DMAs [128, ND, 128] (512B runs)
    ordered by first use: slice m feeds qkT chunk m, slices 12-17 feed V.
    Late slices + W_proj loads are issued from inside the pipeline.
  - QK chunks: qkT[m] = W_qkv[:, m*128:...]^T @ xT (fp32r, tf32 in-PE
    via bitcast - no staging copies).  V: [tok, dv] bf16 + ones column
    per head (AV's 65th output column = softmax denominator).
  - per (head, q-chunk) iteration, software-pipelined across engines:
      scores s = Q K^T (PE, fp32r)
      one DVE tensor_tensor_reduce: ps = (mask - s)*8,
        accum = min(ps) = -8*rowmax  (mask = +1e10 on causal-off entries)
      exp on ACT: w = exp(-ps + accum) -> bf16 (masked entries exp -> 0)
      PE-transpose w blocks; evacuate PSUM->SBUF on ACT/Pool/DVE rotation
      o = w^T-blocks @ [V_h | 1] accumulated (PE, bf16)
      attn[:, h] = o[:, :64] / o[:, 64]  (one Pool divide)
  - last round interleaves heads 10/11 so the output projection
    (attn^T chunks @ W_proj + b) spreads across the round instead of
    piling up at the end.
"""

import contextlib

import numpy as np

import concourse.bass as bass
import concourse.mybir as mybir
import concourse.tile as tile
from concourse import bacc
from concourse.bass_utils import run_bass_kernel_spmd
from concourse.masks import make_causal_mask, make_identity

B, S, D = 8, 1024, 768
H, HD = 12, 64
HV = 65  # V block width per head: 64 value cols + ones col (denominator)
NT = S // 128   # 8 token chunks
ND = D // 128   # 6 d chunks
F32 = mybir.dt.float32
F32R = mybir.dt.float32r
BF16 = mybir.dt.bfloat16

N_CORES = 8
STAGE_LABELS = []   # (label, [instr names]) filled during build when LABELING
LABELING = False
SC_LEAD = 3    # scores lead transposes by this many iterations
PBIG_BUFS = 2
PAV_SHARED = True
SMALL_PS = False
AV_LAG = 2
PTINY_BUFS = 4


def bank_chunks(size):
    out = []
    start = 0
    while start < size:
        end = min(start + 512, size, (start // 512 + 1) * 512)
        out.append((start, end))
        start = end
    return out


def build_mha(nc):
    @contextlib.contextmanager
    def lab(label):
        if not LABELING:
            yield
            return
        before = {i.name for i in nc.all_instructions()}
        yield
        STAGE_LABELS.append(
            (label,
             [i.name for i in nc.all_instructions() if i.name not in before])
        )

    x_d = nc.dram_tensor("x", [S, D], F32, kind="ExternalInput")
    wqkv_d = nc.dram_tensor("W_qkv", [D, 3 * D], F32, kind="ExternalInput")
    wproj_d = nc.dram_tensor("W_proj", [D, D], F32, kind="ExternalInput")
    bproj_d = nc.dram_tensor("b_proj", [1, D], F32, kind="ExternalInput")
    out_d = nc.dram_tensor("out", [S, D], F32, kind="ExternalOutput")

    with tile.TileContext(nc) as tc:
        with (
            tc.tile_pool(name="persist", bufs=1) as pp,
            tc.tile_pool(name="psum", bufs=1, space="PSUM") as psum,
        ):
            def ptile():
                return psum.tile([128, 1024], F32, name="p1", tag="pbig",
                                 bufs=PBIG_BUFS)

            def ptiny():
                return psum.tile([128, 512], F32, name="pt1", tag="ptiny",
                                 bufs=PTINY_BUFS)

            def pavtile():
                if PAV_SHARED:
                    return ptiny()
                return psum.tile([128, 128], F32, name="pav1", tag="pav",
                                 bufs=2)

            # ---- constants (no DMAs yet: x gets the fabric first) ----
            ident_f32 = pp.tile([128, 128], F32, name="ident_f32", tag="ident_f32")
            make_identity(nc, ident_f32[:])
            ident_bf16 = pp.tile([128, 128], BF16, name="ident_bf16", tag="ident_bf16")
            nc.vector.tensor_copy(ident_bf16[:], ident_f32[:])
            ident_f32r = pp.tile([128, 128], F32R, name="ident_f32r", tag="ident_f32r")
            nc.vector.tensor_copy(ident_f32r[:], ident_f32[:])
            # bigmask: [0, S) cols zero; [S, S+128) = causal block with +1e10
            # on masked (k > q) entries.  (mask - s) * 8 -> masked = +8e10,
            # excluded from the min; exp(-(out) + bias) -> exp(-8e10) = 0.
            bigmask = pp.tile([128, 128], F32, name="bigmask", tag="bigmask")
            make_causal_mask(nc, bigmask[:], mask_val=-1e10)

            # ---- persistent activations ----
            qkT = [pp.tile([128, S], F32R, name=f"qkT{m}", tag=f"qkT{m}") for m in range(12)]
            v_sb = [pp.tile([128, H * HV], BF16, name=f"v{qi}", tag=f"v{qi}") for qi in range(NT)]
            attn = [pp.tile([128, D], BF16, name=f"attn{qi}", tag=f"attn{qi}") for qi in range(NT)]
            attnT = pp.tile([128, ND, S], BF16, name="attnT", tag="attnT")
            xT = pp.tile([128, ND, S], F32R, name="xT", tag="xT")
            wp = [pp.tile([128, D], BF16, name=f"wp{di}", tag=f"wp{di}") for di in range(ND)]
            wq = pp.tile([128, ND, 3 * D], F32R, name="wq", tag="wq")
            b_row = pp.tile([1, D], F32R, name="b_row", tag="b_row")
            ones_colf = pp.tile([1, 128], F32, name="ones_colf", tag="ones_colf")
            nc.vector.memset(ones_colf[:], 1.0)
            ones_col = pp.tile([1, 128], F32R, name="ones_col", tag="ones_col")
            nc.vector.tensor_copy(ones_col[:], ones_colf[:])
            b_bcast = pp.tile([128, D], F32, name="b_bcast", tag="b_bcast")

            # W_qkv as 18 column-slice DMAs [128, ND, 128] (512B runs):
            # slice m covers columns m*128..(m+1)*128 of every d-block.
            def wq_slice_dma(eng, m):
                eng.dma_start(
                    wq[:, :, m * 128:(m + 1) * 128],
                    wqkv_d.rearrange("(d p) c -> p d c", p=128)
                    [:, :, m * 128:(m + 1) * 128].bitcast(F32R),
                )

            for qi in range(NT):
                nc.gpsimd.memset(
                    v_sb[qi][:].rearrange("p (h v) -> p h v", v=HV)[:, :, HD:], 1.0
                )

            with (
                tc.tile_pool(name="xpool", bufs=3) as xp,
                tc.tile_pool(name="ph3", bufs=3) as p3,
                tc.tile_pool(name="ph3s", bufs=3) as p3s,
                tc.tile_pool(name="ypool", bufs=1) as yp,
            ):
                # ---- emitters ----
                xevac_eng = [nc.vector, nc.scalar]

                def x_chunk(qi):
                    x_t = xp.tile([128, D], F32R, name="x_t", tag="x_t")
                    nc.sync.dma_start(
                        x_t[:], x_d[qi * 128:(qi + 1) * 128, :].bitcast(F32R)
                    )
                    pxT = ptile()
                    for di in range(ND):
                        nc.tensor.transpose(
                            pxT[:, di * 128:(di + 1) * 128].bitcast(F32R),
                            x_t[:, di * 128:(di + 1) * 128],
                            ident_f32r[:],
                        )
                    eng = xevac_eng[qi % 2]
                    dst = xT[:, :, qi * 128:(qi + 1) * 128]
                    src = pxT[:, :D].rearrange("p (d q) -> p d q", q=128)
                    if eng is nc.scalar:
                        eng.copy(dst, src)
                    else:
                        eng.tensor_copy(dst, src)

                def v_proj(qi):
                    pv = ptile()
                    for c0, c1 in bank_chunks(D):
                        for di in range(ND):
                            nc.tensor.matmul(
                                pv[:, c0:c1],
                                xT[:, di, qi * 128:(qi + 1) * 128],
                                wq[:, di, 2 * D + c0:2 * D + c1],
                                start=(di == 0), stop=(di == ND - 1),
                            )
                    dst = v_sb[qi][:].rearrange("p (h v) -> p h v", v=HV)[:, :, :HD]
                    srcv = pv[:, :D].rearrange("p (h v) -> p h v", v=HD)
                    if qi % 2 == 0:
                        nc.vector.tensor_copy(dst, srcv)
                    else:
                        nc.scalar.copy(dst, srcv)

                def qk_half(m, half):
                    pqk = ptiny()
                    c0, c1 = (0, 512) if half == 0 else (512, 1024)
                    for di in range(ND):
                        nc.tensor.matmul(
                            pqk[:, 0:512],
                            wq[:, di, m * 128:(m + 1) * 128],
                            xT[:, di, c0:c1],
                            start=(di == 0), stop=(di == ND - 1),
                        )
                    if (m + half) % 2 == 0:
                        if m < 6:
                            nc.vector.tensor_scalar_mul(
                                qkT[m][:, c0:c1], pqk[:, 0:512], 8.0)
                        else:
                            nc.vector.tensor_copy(qkT[m][:, c0:c1], pqk[:, 0:512])
                    else:
                        if m < 6:
                            nc.scalar.mul(qkT[m][:, c0:c1], pqk[:, 0:512], 8.0)
                        else:
                            nc.scalar.copy(qkT[m][:, c0:c1], pqk[:, 0:512])

                def emit_b_bcast():
                    for c0, c1 in bank_chunks(D):
                        pb = ptiny()
                        nc.tensor.matmul(
                            pb[:, 0:c1 - c0], ones_col[:],
                            b_row[:, c0:c1],
                            start=True, stop=True,
                        )
                        nc.vector.tensor_copy(
                            b_bcast[:, c0:c1], pb[:, 0:c1 - c0]
                        )

                # ---- prologue ----
                wq_slice_dma(nc.scalar, 0)
                wq_slice_dma(nc.scalar, 6)
                with lab("xphase"):
                    for qi in range(4):
                        x_chunk(qi)
                with lab("qk0a"):
                    qk_half(0, 0)
                with lab("qk6a"):
                    qk_half(6, 0)
                with lab("xphase2"):
                    for qi in range(4, NT):
                        x_chunk(qi)
                # V slices + later Q/K slices + W_proj: held back with
                # clock waits so the greedy scheduler cannot let them steal
                # serial DMA bandwidth from x / m0 / m6.
                with tc.tile_wait_until(0.009):
                    for m in (12, 13, 14, 15, 16, 17):
                        wq_slice_dma(nc.scalar, m)
                with tc.tile_wait_until(0.016):
                    nc.scalar.dma_start(b_row[:], bproj_d[:].bitcast(F32R))
                for i, (ma, mb) in enumerate(
                        ((1, 7), (2, 8), (3, 9), (4, 10), (5, 11))):
                    with tc.tile_wait_until(0.017 + 0.003 * i):
                        wq_slice_dma(nc.scalar, ma)
                        wq_slice_dma(nc.scalar, mb)
                for di in range(ND):
                    with tc.tile_wait_until(0.032 + 0.001 * di):
                        # SWDGE cast fp32 -> bf16; needed only by proj
                        nc.gpsimd.dma_start(
                            wp[di][:],
                            wproj_d[di * 128:(di + 1) * 128, :],
                        )
                with lab("qk6b"):
                    qk_half(6, 1)
                with lab("qk0b"):
                    qk_half(0, 1)

                # ---- iteration order: heads 0..9 q-major, then last round
                # interleaves heads 10/11 so proj spreads over the round ----
                iters = [(h, qi) for h in range(10) for qi in range(NT)]
                for qi in range(NT):
                    iters.append((10, qi))
                    iters.append((11, qi))

                # step -> late weight DMAs / deferred setup
                dma_sched = {36: ("bbc", 0)}
                # step -> PE filler (V projections early, QK halves after)
                fill_sched = {}
                for qi in range(NT):
                    fill_sched[qi] = ("v", qi)
                _i = 0
                for m in (1, 7, 2, 8, 3, 9, 4, 10, 5, 11):
                    for half in (0, 1):
                        fill_sched[8 + 2 * _i] = ("qk", (m, half))
                        _i += 1

                # ---- attention software pipeline ----
                st = {}  # t -> dict of live tiles for iteration t

                def em_scores(t):
                    h, qi = iters[t]
                    qoff = (h % 2) * 64
                    Qt = qkT[h // 2]
                    Kt = qkT[6 + h // 2]
                    ks = (qi + 1) * 128
                    ps = ptiny() if (SMALL_PS and qi <= 3) else ptile()
                    lhs = Qt[qoff:qoff + 64, qi * 128:(qi + 1) * 128]
                    for c0, c1 in bank_chunks(ks):
                        nc.tensor.matmul(
                            ps[:, c0:c1], lhs, Kt[qoff:qoff + 64, c0:c1],
                            start=True, stop=True,
                        )
                    st[t] = {"ps": ps}

                def em_maskmax(t):
                    h, qi = iters[t]
                    ks = (qi + 1) * 128
                    ps = st[t]["ps"]
                    nc.vector.tensor_tensor(
                        out=ps[:, qi * 128:ks],
                        in0=ps[:, qi * 128:ks],
                        in1=bigmask[:],
                        op=mybir.AluOpType.add,
                    )
                    neg8m = p3s.tile([128, 1], F32, name="neg8m", tag="neg8m", bufs=6)
                    nc.vector.reduce_max(
                        out=neg8m[:], in_=ps[:, :ks],
                        axis=mybir.AxisListType.X, negate=True,
                    )
                    st[t]["neg8m"] = neg8m

                def em_exp(t):
                    h, qi = iters[t]
                    ks = (qi + 1) * 128
                    w_t = p3s.tile([128, S], BF16, name="w_t", tag="w_t")
                    nc.scalar.activation(
                        w_t[:, :ks], st[t]["ps"][:, :ks],
                        mybir.ActivationFunctionType.Exp,
                        bias=st[t]["neg8m"][:], scale=1.0,
                    )
                    st[t]["w_t"] = w_t

                def em_transp(t):
                    h, qi = iters[t]
                    w_t = st[t]["w_t"]
                    pt8 = ptiny().bitcast(BF16).rearrange("p (b q) -> p b q", q=128)
                    for ki in range(qi + 1):
                        nc.tensor.transpose(
                            pt8[:, ki, :],
                            w_t[:, ki * 128:(ki + 1) * 128],
                            ident_bf16[:],
                        )
                    st[t]["pt8"] = pt8

                def em_wevac(t):
                    h, qi = iters[t]
                    wT = p3.tile([128, NT, 128], BF16, name="wT", tag="wT")
                    eng = (nc.scalar, nc.vector)[t % 2]
                    if eng is nc.scalar:
                        eng.copy(wT[:, :qi + 1, :], st[t]["pt8"][:, :qi + 1, :])
                    else:
                        eng.tensor_copy(wT[:, :qi + 1, :], st[t]["pt8"][:, :qi + 1, :])
                    st[t]["wT"] = wT

                def em_av(t):
                    h, qi = iters[t]
                    po = pavtile()
                    for ki in range(qi + 1):
                        nc.tensor.matmul(
                            po[:, :HV],
                            st[t]["wT"][:, ki, :],
                            v_sb[ki][:, h * HV:(h + 1) * HV],
                            start=(ki == 0), stop=(ki == qi),
                        )
                    st[t]["po"] = po

                def em_scale(t):
                    h, qi = iters[t]
                    po = st[t]["po"]
                    recip = p3s.tile([128, 1], F32, name="recip", tag="recip",
                                     bufs=6)
                    nc.vector.reciprocal(recip[:], po[:, HD:HV])
                    dst = attn[qi][:, h * HD:(h + 1) * HD]
                    if t % 2 == 0:
                        nc.vector.tensor_scalar_mul(dst, po[:, :HD], recip[:])
                    else:
                        nc.scalar.mul(dst, po[:, :HD], recip[:])
                    del st[t]

                def em_proj_a(qi):
                    pat = ptiny().bitcast(BF16).rearrange("p (b q) -> p b q", q=128)
                    for di in range(ND):
                        nc.tensor.transpose(
                            pat[:, di, :],
                            attn[qi][:, di * 128:(di + 1) * 128],
                            ident_bf16[:],
                        )
                    nc.scalar.copy(
                        attnT[:, :, qi * 128:(qi + 1) * 128], pat[:, :ND, :]
                    )

                def em_proj_b(qi):
                    y_t = yp.tile([128, D], F32, name="y_t", tag="y_t")
                    for c0, c1 in bank_chunks(D):
                        py = ptiny()
                        for di in range(ND):
                            nc.tensor.matmul(
                                py[:, 0:c1 - c0],
                                attnT[:, di, qi * 128:(qi + 1) * 128],
                                wp[di][:, c0:c1],
                                start=(di == 0), stop=(di == ND - 1),
                            )
                        nc.vector.tensor_tensor(
                            out=y_t[:, c0:c1], in0=py[:, 0:c1 - c0],
                            in1=b_bcast[:, c0:c1],
                            op=mybir.AluOpType.add,
                        )
                    nc.sync.dma_start(
                        out_d[qi * 128:(qi + 1) * 128, :], y_t[:]
                    )

                NIT = 12 * NT  # 96
                for s_step in range(NIT + SC_LEAD + AV_LAG + 4):
                    t_sc = s_step - SC_LEAD - AV_LAG - 1
                    if 0 <= t_sc < NIT:
                        with lab(f"scale{t_sc}"):
                            em_scale(t_sc)
                    t_pa = s_step - SC_LEAD - AV_LAG - 2
                    if 0 <= t_pa < NIT and iters[t_pa][0] == H - 1:
                        with lab(f"proja{iters[t_pa][1]}"):
                            em_proj_a(iters[t_pa][1])
                    t_pb = s_step - SC_LEAD - AV_LAG - 3
                    if 0 <= t_pb < NIT and iters[t_pb][0] == H - 1:
                        with lab(f"projb{iters[t_pb][1]}"):
                            em_proj_b(iters[t_pb][1])
                    if s_step < NIT:
                        with lab(f"scores{s_step}"):
                            em_scores(s_step)
                        with lab(f"maskmax{s_step}"):
                            em_maskmax(s_step)
                        with lab(f"exp{s_step}"):
                            em_exp(s_step)
                    t_tr = s_step - SC_LEAD
                    if 0 <= t_tr < NIT:
                        with lab(f"transp{t_tr}"):
                            em_transp(t_tr)
                        with lab(f"wevac{t_tr}"):
                            em_wevac(t_tr)
                    t_av = s_step - SC_LEAD - AV_LAG
                    if 0 <= t_av < NIT:
                        with lab(f"av{t_av}"):
                            em_av(t_av)
                    if dma_sched.get(s_step) is not None:
                        with lab("bbcast"):
                            emit_b_bcast()
                    fs = fill_sched.get(s_step)
                    if fs is not None:
                        kind, arg = fs
                        with lab(f"fill_{kind}{arg}"):
                            if kind == "v":
                                v_proj(arg)
                            else:
                                qk_half(*arg)

    nc.compile()
    return nc


_NC_CACHE = None


def _get_nc():
    global _NC_CACHE
    if _NC_CACHE is None:
        nc = bacc.Bacc(
            "TRN2",
            target_bir_lowering=False,
            debug=False,
            num_devices=N_CORES,
        )
        build_mha(nc)
        _NC_CACHE = nc
    return _NC_CACHE


def kernel(x, W_qkv, W_proj, b_proj):
    nc = _get_nc()
    x = np.ascontiguousarray(np.asarray(x, dtype=np.float32))
    W_qkv = np.ascontiguousarray(np.asarray(W_qkv, dtype=np.float32))
    W_proj = np.ascontiguousarray(np.asarray(W_proj, dtype=np.float32))
    b_proj = np.ascontiguousarray(
        np.asarray(b_proj, dtype=np.float32).reshape(1, D)
    )
    in_maps = [
        {"x": x[b], "W_qkv": W_qkv, "W_proj": W_proj, "b_proj": b_proj}
        for b in range(N_CORES)
    ]
    res = run_bass_kernel_spmd(nc, in_maps, core_ids=list(range(N_CORES)))
    return np.stack([res.results[b]["out"] for b in range(N_CORES)], axis=0)


# revision 8
# speedup vs baseline: 1.1968x; 1.0069x over previous
"""Multi-head attention (B=8, S=1024, D=768, H=12) on 8 TRN2 NeuronCores.

Sharding: pure batch parallelism - one batch element per core, weights
replicated. No collectives.

Per-core structure (tokens S=1024, D=768, H=12 heads of HD=64):
  - x is DMA'd first (the DMA fabric is the prologue's serial resource),
    PE-transposed to xT [d, tok] via a bf16 identity (1 cyc/row).
  - W_qkv lands as 18 column-slice DMAs [128, ND, 128] (512B runs)
    ordered by first use: slice m feeds qkT chunk m, slices 12-17 feed V.
    Late slices + W_proj loads are issued from inside the pipeline.
  - QK chunks: qkT[m] = W_qkv[:, m*128:...]^T @ xT (fp32r, tf32 in-PE
    via bitcast - no staging copies).  V: [tok, dv] bf16 + ones column
    per head (AV's 65th output column = softmax denominator).
  - per (head, q-chunk) iteration, software-pipelined across engines:
      scores s = Q K^T (PE, fp32r)
      one DVE tensor_tensor_reduce: ps = (mask - s)*8,
        accum = min(ps) = -8*rowmax  (mask = +1e10 on causal-off entries)
      exp on ACT: w = exp(-ps + accum) -> bf16 (masked entries exp -> 0)
      PE-transpose w blocks; evacuate PSUM->SBUF on ACT/Pool/DVE rotation
      o = w^T-blocks @ [V_h | 1] accumulated (PE, bf16)
      attn[:, h] = o[:, :64] / o[:, 64]  (one Pool divide)
  - last round interleaves heads 10/11 so the output projection
    (attn^T chunks @ W_proj + b) spreads across the round instead of
    piling up at the end.
"""

import contextlib

import numpy as np

import concourse.bass as bass
import concourse.mybir as mybir
import concourse.tile as tile
from concourse import bacc
from concourse.bass_utils import run_bass_kernel_spmd
from concourse.masks import make_causal_mask, make_identity

B, S, D = 8, 1024, 768
H, HD = 12, 64
HV = 65  # V block width per head: 64 value cols + ones col (denominator)
NT = S // 128   # 8 token chunks
ND = D // 128   # 6 d chunks
F32 = mybir.dt.float32
F32R = mybir.dt.float32r
BF16 = mybir.dt.bfloat16

N_CORES = 8
STAGE_LABELS = []   # (label, [instr names]) filled during build when LABELING
LABELING = False
SC_LEAD = 3    # scores lead transposes by this many iterations
PBIG_BUFS = 2
PAV_SHARED = True
SMALL_PS = False
AV_LAG = 3
WEVAC_PAT = "AD"     # cycle of engines for wT evacuation (A=ACT, D=DVE)
SCALE_PAT = "D"     # cycle for the o/denominator scaled copy
PTINY_BUFS = 4


def bank_chunks(size):
    out = []
    start = 0
    while start < size:
        end = min(start + 512, size, (start // 512 + 1) * 512)
        out.append((start, end))
        start = end
    return out


def build_mha(nc):
    @contextlib.contextmanager
    def lab(label):
        if not LABELING:
            yield
            return
        before = {i.name for i in nc.all_instructions()}
        yield
        STAGE_LABELS.append(
            (label,
             [i.name for i in nc.all_instructions() if i.name not in before])
        )

    x_d = nc.dram_tensor("x", [S, D], F32, kind="ExternalInput")
    wqkv_d = nc.dram_tensor("W_qkv", [D, 3 * D], F32, kind="ExternalInput")
    wproj_d = nc.dram_tensor("W_proj", [D, D], F32, kind="ExternalInput")
    bproj_d = nc.dram_tensor("b_proj", [1, D], F32, kind="ExternalInput")
    out_d = nc.dram_tensor("out", [S, D], F32, kind="ExternalOutput")

    with tile.TileContext(nc) as tc:
        with (
            tc.tile_pool(name="persist", bufs=1) as pp,
            tc.tile_pool(name="psum", bufs=1, space="PSUM") as psum,
        ):
            def ptile():
                return psum.tile([128, 1024], F32, name="p1", tag="pbig",
                                 bufs=PBIG_BUFS)

            def ptiny():
                return psum.tile([128, 512], F32, name="pt1", tag="ptiny",
                                 bufs=PTINY_BUFS)

            def pavtile():
                if PAV_SHARED:
                    return ptiny()
                return psum.tile([128, 128], F32, name="pav1", tag="pav",
                                 bufs=2)

            # ---- constants (no DMAs yet: x gets the fabric first) ----
            ident_f32 = pp.tile([128, 128], F32, name="ident_f32", tag="ident_f32")
            make_identity(nc, ident_f32[:])
            ident_bf16 = pp.tile([128, 128], BF16, name="ident_bf16", tag="ident_bf16")
            nc.vector.tensor_copy(ident_bf16[:], ident_f32[:])
            ident_f32r = pp.tile([128, 128], F32R, name="ident_f32r", tag="ident_f32r")
            nc.vector.tensor_copy(ident_f32r[:], ident_f32[:])
            # bigmask: [0, S) cols zero; [S, S+128) = causal block with +1e10
            # on masked (k > q) entries.  (mask - s) * 8 -> masked = +8e10,
            # excluded from the min; exp(-(out) + bias) -> exp(-8e10) = 0.
            bigmask = pp.tile([128, 128], F32, name="bigmask", tag="bigmask")
            make_causal_mask(nc, bigmask[:], mask_val=-1e10)
            # transposed bf16 copy for PE mask-preload (lhsT so that
            # matmul(maskT, ident) writes bigmask into PSUM)
            bigmask_bf = pp.tile([128, 128], BF16, name="bigmask_bf",
                                 tag="bigmask_bf")
            nc.vector.tensor_copy(bigmask_bf[:], bigmask[:])
            maskT_bf = pp.tile([128, 128], BF16, name="maskT_bf", tag="maskT_bf")

            # ---- persistent activations ----
            qkT = [pp.tile([128, S], F32R, name=f"qkT{m}", tag=f"qkT{m}") for m in range(12)]
            v_sb = [pp.tile([128, H * HV], BF16, name=f"v{qi}", tag=f"v{qi}") for qi in range(NT)]
            attn = [pp.tile([128, D], BF16, name=f"attn{qi}", tag=f"attn{qi}") for qi in range(NT)]
            attnT = pp.tile([128, ND, S], BF16, name="attnT", tag="attnT")
            xT = pp.tile([128, ND, S], F32R, name="xT", tag="xT")
            wp = [pp.tile([128, D], BF16, name=f"wp{di}", tag=f"wp{di}") for di in range(ND)]
            wq = pp.tile([128, ND, 3 * D], F32R, name="wq", tag="wq")
            b_row = pp.tile([1, D], F32R, name="b_row", tag="b_row")
            ones_colf = pp.tile([1, 128], F32, name="ones_colf", tag="ones_colf")
            nc.vector.memset(ones_colf[:], 1.0)
            ones_col = pp.tile([1, 128], F32R, name="ones_col", tag="ones_col")
            nc.vector.tensor_copy(ones_col[:], ones_colf[:])
            b_bcast = pp.tile([128, D], F32, name="b_bcast", tag="b_bcast")

            # W_qkv as 18 column-slice DMAs [128, ND, 128] (512B runs):
            # slice m covers columns m*128..(m+1)*128 of every d-block.
            def wq_slice_dma(eng, m):
                eng.dma_start(
                    wq[:, :, m * 128:(m + 1) * 128],
                    wqkv_d.rearrange("(d p) c -> p d c", p=128)
                    [:, :, m * 128:(m + 1) * 128].bitcast(F32R),
                )

            for qi in range(NT):
                nc.gpsimd.memset(
                    v_sb[qi][:].rearrange("p (h v) -> p h v", v=HV)[:, :, HD:], 1.0
                )

            with (
                tc.tile_pool(name="xpool", bufs=3) as xp,
                tc.tile_pool(name="ph3", bufs=3) as p3,
                tc.tile_pool(name="ph3s", bufs=3) as p3s,
                tc.tile_pool(name="ypool", bufs=1) as yp,
            ):
                # ---- emitters ----
                xevac_eng = [nc.vector, nc.scalar]

                def x_chunk(qi):
                    x_t = xp.tile([128, D], F32R, name="x_t", tag="x_t")
                    nc.sync.dma_start(
                        x_t[:], x_d[qi * 128:(qi + 1) * 128, :].bitcast(F32R)
                    )
                    pxT = ptile()
                    for di in range(ND):
                        nc.tensor.transpose(
                            pxT[:, di * 128:(di + 1) * 128].bitcast(F32R),
                            x_t[:, di * 128:(di + 1) * 128],
                            ident_f32r[:],
                        )
                    eng = xevac_eng[qi % 2]
                    dst = xT[:, :, qi * 128:(qi + 1) * 128]
                    src = pxT[:, :D].rearrange("p (d q) -> p d q", q=128)
                    if eng is nc.scalar:
                        eng.copy(dst, src)
                    else:
                        eng.tensor_copy(dst, src)

                def v_proj(qi):
                    pv = ptile()
                    for c0, c1 in bank_chunks(D):
                        for di in range(ND):
                            nc.tensor.matmul(
                                pv[:, c0:c1],
                                xT[:, di, qi * 128:(qi + 1) * 128],
                                wq[:, di, 2 * D + c0:2 * D + c1],
                                start=(di == 0), stop=(di == ND - 1),
                            )
                    dst = v_sb[qi][:].rearrange("p (h v) -> p h v", v=HV)[:, :, :HD]
                    srcv = pv[:, :D].rearrange("p (h v) -> p h v", v=HD)
                    if qi % 2 == 0:
                        nc.vector.tensor_copy(dst, srcv)
                    else:
                        nc.scalar.copy(dst, srcv)

                def qk_half(m, half):
                    pqk = ptiny()
                    c0, c1 = (0, 512) if half == 0 else (512, 1024)
                    for di in range(ND):
                        nc.tensor.matmul(
                            pqk[:, 0:512],
                            wq[:, di, m * 128:(m + 1) * 128],
                            xT[:, di, c0:c1],
                            start=(di == 0), stop=(di == ND - 1),
                        )
                    if (m + half) % 2 == 0:
                        if m < 6:
                            nc.vector.tensor_scalar_mul(
                                qkT[m][:, c0:c1], pqk[:, 0:512], 8.0)
                        else:
                            nc.vector.tensor_copy(qkT[m][:, c0:c1], pqk[:, 0:512])
                    else:
                        if m < 6:
                            nc.scalar.mul(qkT[m][:, c0:c1], pqk[:, 0:512], 8.0)
                        else:
                            nc.scalar.copy(qkT[m][:, c0:c1], pqk[:, 0:512])

                def emit_b_bcast():
                    for c0, c1 in bank_chunks(D):
                        pb = ptiny()
                        nc.tensor.matmul(
                            pb[:, 0:c1 - c0], ones_col[:],
                            b_row[:, c0:c1],
                            start=True, stop=True,
                        )
                        nc.vector.tensor_copy(
                            b_bcast[:, c0:c1], pb[:, 0:c1 - c0]
                        )

                # ---- prologue ----
                ptm = ptiny().bitcast(BF16)
                nc.tensor.transpose(ptm[:, 0:128], bigmask_bf[:], ident_bf16[:])
                nc.vector.tensor_copy(maskT_bf[:], ptm[:, 0:128])
                wq_slice_dma(nc.scalar, 0)
                wq_slice_dma(nc.scalar, 6)
                with lab("xphase"):
                    for qi in range(4):
                        x_chunk(qi)
                with lab("qk0a"):
                    qk_half(0, 0)
                with lab("qk6a"):
                    qk_half(6, 0)
                with lab("xphase2"):
                    for qi in range(4, NT):
                        x_chunk(qi)
                # V slices + later Q/K slices + W_proj: held back with
                # clock waits so the greedy scheduler cannot let them steal
                # serial DMA bandwidth from x / m0 / m6.
                with tc.tile_wait_until(0.009):
                    for m in (12, 13, 14, 15, 16, 17):
                        wq_slice_dma(nc.scalar, m)
                with tc.tile_wait_until(0.016):
                    nc.scalar.dma_start(b_row[:], bproj_d[:].bitcast(F32R))
                for i, (ma, mb) in enumerate(
                        ((1, 7), (2, 8), (3, 9), (4, 10), (5, 11))):
                    with tc.tile_wait_until(0.017 + 0.003 * i):
                        wq_slice_dma(nc.scalar, ma)
                        wq_slice_dma(nc.scalar, mb)
                for di in range(ND):
                    with tc.tile_wait_until(0.032 + 0.001 * di):
                        # SWDGE cast fp32 -> bf16; needed only by proj
                        nc.gpsimd.dma_start(
                            wp[di][:],
                            wproj_d[di * 128:(di + 1) * 128, :],
                        )
                with lab("qk6b"):
                    qk_half(6, 1)
                with lab("qk0b"):
                    qk_half(0, 1)

                # ---- iteration order: heads 0..9 q-major, then last round
                # interleaves heads 10/11 so proj spreads over the round ----
                iters = [(h, qi) for h in range(10) for qi in range(NT)]
                for qi in range(NT):
                    iters.append((10, qi))
                    iters.append((11, qi))

                # step -> late weight DMAs / deferred setup
                dma_sched = {36: ("bbc", 0)}
                # step -> PE filler (V projections early, QK halves after)
                fill_sched = {}
                for qi in range(NT):
                    fill_sched[qi] = ("v", qi)
                _i = 0
                for m in (1, 7, 2, 8, 3, 9, 4, 10, 5, 11):
                    for half in (0, 1):
                        fill_sched[8 + 2 * _i] = ("qk", (m, half))
                        _i += 1

                # ---- attention software pipeline ----
                st = {}  # t -> dict of live tiles for iteration t

                def em_scores(t):
                    h, qi = iters[t]
                    qoff = (h % 2) * 64
                    Qt = qkT[h // 2]
                    Kt = qkT[6 + h // 2]
                    ks = (qi + 1) * 128
                    ps = ptiny() if (SMALL_PS and qi <= 3) else ptile()
                    lhs = Qt[qoff:qoff + 64, qi * 128:(qi + 1) * 128]
                    pe_mask = qi in (0, 4)
                    for c0, c1 in bank_chunks(ks):
                        if pe_mask and c0 == qi * 128:
                            # diag block is exactly this bank chunk: write
                            # the causal mask via PE, accumulate scores on it
                            nc.tensor.matmul(
                                ps[:, c0:c1], maskT_bf[:], ident_bf16[:],
                                start=True, stop=True, skip_group_check=True,
                            )
                            nc.tensor.matmul(
                                ps[:, c0:c1], lhs, Kt[qoff:qoff + 64, c0:c1],
                                start=False, stop=True, skip_group_check=True,
                            )
                        else:
                            nc.tensor.matmul(
                                ps[:, c0:c1], lhs, Kt[qoff:qoff + 64, c0:c1],
                                start=True, stop=True,
                            )
                    st[t] = {"ps": ps, "pe_mask": pe_mask}

                def em_maskmax(t):
                    h, qi = iters[t]
                    ks = (qi + 1) * 128
                    ps = st[t]["ps"]
                    if not st[t]["pe_mask"]:
                        nc.vector.tensor_tensor(
                            out=ps[:, qi * 128:ks],
                            in0=ps[:, qi * 128:ks],
                            in1=bigmask[:],
                            op=mybir.AluOpType.add,
                        )
                    neg8m = p3s.tile([128, 1], F32, name="neg8m", tag="neg8m", bufs=6)
                    nc.vector.reduce_max(
                        out=neg8m[:], in_=ps[:, :ks],
                        axis=mybir.AxisListType.X, negate=True,
                    )
                    st[t]["neg8m"] = neg8m

                def em_exp(t):
                    h, qi = iters[t]
                    ks = (qi + 1) * 128
                    w_t = p3s.tile([128, S], BF16, name="w_t", tag="w_t")
                    nc.scalar.activation(
                        w_t[:, :ks], st[t]["ps"][:, :ks],
                        mybir.ActivationFunctionType.Exp,
                        bias=st[t]["neg8m"][:], scale=1.0,
                    )
                    st[t]["w_t"] = w_t

                def em_transp(t):
                    h, qi = iters[t]
                    w_t = st[t]["w_t"]
                    pt8 = ptiny().bitcast(BF16).rearrange("p (b q) -> p b q", q=128)
                    for ki in range(qi + 1):
                        nc.tensor.transpose(
                            pt8[:, ki, :],
                            w_t[:, ki * 128:(ki + 1) * 128],
                            ident_bf16[:],
                        )
                    st[t]["pt8"] = pt8

                def em_wevac(t):
                    h, qi = iters[t]
                    wT = p3.tile([128, NT, 128], BF16, name="wT", tag="wT")
                    eng = nc.scalar if WEVAC_PAT[t % len(WEVAC_PAT)] == 'A' else nc.vector
                    if eng is nc.scalar:
                        eng.copy(wT[:, :qi + 1, :], st[t]["pt8"][:, :qi + 1, :])
                    else:
                        eng.tensor_copy(wT[:, :qi + 1, :], st[t]["pt8"][:, :qi + 1, :])
                    st[t]["wT"] = wT

                def em_av(t):
                    h, qi = iters[t]
                    po = pavtile()
                    for ki in range(qi + 1):
                        nc.tensor.matmul(
                            po[:, :HV],
                            st[t]["wT"][:, ki, :],
                            v_sb[ki][:, h * HV:(h + 1) * HV],
                            start=(ki == 0), stop=(ki == qi),
                        )
                    st[t]["po"] = po

                def em_scale(t):
                    h, qi = iters[t]
                    po = st[t]["po"]
                    recip = p3s.tile([128, 1], F32, name="recip", tag="recip",
                                     bufs=6)
                    nc.vector.reciprocal(recip[:], po[:, HD:HV])
                    dst = attn[qi][:, h * HD:(h + 1) * HD]
                    if SCALE_PAT[t % len(SCALE_PAT)] == 'A':
                        nc.scalar.mul(dst, po[:, :HD], recip[:])
                    else:
                        nc.vector.tensor_scalar_mul(dst, po[:, :HD], recip[:])
                    del st[t]

                def em_proj_a(qi):
                    pat = ptiny().bitcast(BF16).rearrange("p (b q) -> p b q", q=128)
                    for di in range(ND):
                        nc.tensor.transpose(
                            pat[:, di, :],
                            attn[qi][:, di * 128:(di + 1) * 128],
                            ident_bf16[:],
                        )
                    nc.scalar.copy(
                        attnT[:, :, qi * 128:(qi + 1) * 128], pat[:, :ND, :]
                    )

                def em_proj_b(qi):
                    y_t = yp.tile([128, D], F32, name="y_t", tag="y_t")
                    for c0, c1 in bank_chunks(D):
                        py = ptiny()
                        for di in range(ND):
                            nc.tensor.matmul(
                                py[:, 0:c1 - c0],
                                attnT[:, di, qi * 128:(qi + 1) * 128],
                                wp[di][:, c0:c1],
                                start=(di == 0), stop=(di == ND - 1),
                            )
                        nc.vector.tensor_tensor(
                            out=y_t[:, c0:c1], in0=py[:, 0:c1 - c0],
                            in1=b_bcast[:, c0:c1],
                            op=mybir.AluOpType.add,
                        )
                    nc.sync.dma_start(
                        out_d[qi * 128:(qi + 1) * 128, :], y_t[:]
                    )

                NIT = 12 * NT  # 96
                for s_step in range(NIT + SC_LEAD + AV_LAG + 4):
                    t_sc = s_step - SC_LEAD - AV_LAG - 1
                    if 0 <= t_sc < NIT:
                        with lab(f"scale{t_sc}"):
                            em_scale(t_sc)
                    t_pa = s_step - SC_LEAD - AV_LAG - 2
                    if 0 <= t_pa < NIT and iters[t_pa][0] == H - 1:
                        with lab(f"proja{iters[t_pa][1]}"):
                            em_proj_a(iters[t_pa][1])
                    t_pb = s_step - SC_LEAD - AV_LAG - 3
                    if 0 <= t_pb < NIT and iters[t_pb][0] == H - 1:
                        with lab(f"projb{iters[t_pb][1]}"):
                            em_proj_b(iters[t_pb][1])
                    if s_step < NIT:
                        with lab(f"scores{s_step}"):
                            em_scores(s_step)
                        with lab(f"maskmax{s_step}"):
                            em_maskmax(s_step)
                        with lab(f"exp{s_step}"):
                            em_exp(s_step)
                    t_tr = s_step - SC_LEAD
                    if 0 <= t_tr < NIT:
                        with lab(f"transp{t_tr}"):
                            em_transp(t_tr)
                        with lab(f"wevac{t_tr}"):
                            em_wevac(t_tr)
                    t_av = s_step - SC_LEAD - AV_LAG
                    if 0 <= t_av < NIT:
                        with lab(f"av{t_av}"):
                            em_av(t_av)
                    if dma_sched.get(s_step) is not None:
                        with lab("bbcast"):
                            emit_b_bcast()
                    fs = fill_sched.get(s_step)
                    if fs is not None:
                        kind, arg = fs
                        with lab(f"fill_{kind}{arg}"):
                            if kind == "v":
                                v_proj(arg)
                            else:
                                qk_half(*arg)

    nc.compile()
    return nc


_NC_CACHE = None


def _get_nc():
    global _NC_CACHE
    if _NC_CACHE is None:
        nc = bacc.Bacc(
            "TRN2",
            target_bir_lowering=False,
            debug=False,
            num_devices=N_CORES,
        )
        build_mha(nc)
        _NC_CACHE = nc
    return _NC_CACHE


def kernel(x, W_qkv, W_proj, b_proj):
    nc = _get_nc()
    x = np.ascontiguousarray(np.asarray(x, dtype=np.float32))
    W_qkv = np.ascontiguousarray(np.asarray(W_qkv, dtype=np.float32))
    W_proj = np.ascontiguousarray(np.asarray(W_proj, dtype=np.float32))
    b_proj = np.ascontiguousarray(
        np.asarray(b_proj, dtype=np.float32).reshape(1, D)
    )
    in_maps = [
        {"x": x[b], "W_qkv": W_qkv, "W_proj": W_proj, "b_proj": b_proj}
        for b in range(N_CORES)
    ]
    res = run_bass_kernel_spmd(nc, in_maps, core_ids=list(range(N_CORES)))
    return np.stack([res.results[b]["out"] for b in range(N_CORES)], axis=0)


# revision 9
# speedup vs baseline: 1.2354x; 1.0322x over previous
"""Multi-head attention (B=8, S=1024, D=768, H=12) on 8 TRN2 NeuronCores.

Sharding: pure batch parallelism - one batch element per core, weights
replicated. No collectives.

Per-core structure (tokens S=1024, D=768, H=12 heads of HD=64):
  - x is DMA'd first (the DMA fabric is the prologue's serial resource),
    PE-transposed to xT [d, tok] via a bf16 identity (1 cyc/row).
  - W_qkv lands as 18 column-slice DMAs [128, ND, 128] (512B runs)
    ordered by first use: slice m feeds qkT chunk m, slices 12-17 feed V.
    Late slices + W_proj loads are issued from inside the pipeline.
  - QK chunks: qkT[m] = W_qkv[:, m*128:...]^T @ xT (fp32r, tf32 in-PE
    via bitcast - no staging copies).  V: [tok, dv] bf16 + ones column
    per head (AV's 65th output column = softmax denominator).
  - per (head, q-chunk) iteration, software-pipelined across engines:
      scores s = Q K^T (PE, fp32r)
      one DVE tensor_tensor_reduce: ps = (mask - s)*8,
        accum = min(ps) = -8*rowmax  (mask = +1e10 on causal-off entries)
      exp on ACT: w = exp(-ps + accum) -> bf16 (masked entries exp -> 0)
      PE-transpose w blocks; evacuate PSUM->SBUF on ACT/Pool/DVE rotation
      o = w^T-blocks @ [V_h | 1] accumulated (PE, bf16)
      attn[:, h] = o[:, :64] / o[:, 64]  (one Pool divide)
  - last round interleaves heads 10/11 so the output projection
    (attn^T chunks @ W_proj + b) spreads across the round instead of
    piling up at the end.
"""

import contextlib

import numpy as np

import concourse.bass as bass
import concourse.mybir as mybir
import concourse.tile as tile
from concourse import bacc
from concourse.bass_utils import run_bass_kernel_spmd
from concourse.masks import make_causal_mask, make_identity

B, S, D = 8, 1024, 768
H, HD = 12, 64
HV = 65  # V block width per head: 64 value cols + ones col (denominator)
NT = S // 128   # 8 token chunks
ND = D // 128   # 6 d chunks
F32 = mybir.dt.float32
F32R = mybir.dt.float32r
BF16 = mybir.dt.bfloat16

N_CORES = 8
STAGE_LABELS = []   # (label, [instr names]) filled during build when LABELING
LABELING = False
SC_LEAD = 3    # scores lead transposes by this many iterations
PBIG_BUFS = 2
PAV_SHARED = True
SMALL_PS = False
AV_LAG = 2
WEVAC_PAT = "AD"     # cycle of engines for wT evacuation (A=ACT, D=DVE)
SCALE_PAT = "D"     # cycle for the o/denominator scaled copy
PE_MASK_QIS = (0, 2, 4, 6)  # q-chunks whose causal mask is PE-preloaded into PSUM
PTINY_BUFS = 4


def bank_chunks(size):
    out = []
    start = 0
    while start < size:
        end = min(start + 512, size, (start // 512 + 1) * 512)
        out.append((start, end))
        start = end
    return out


def build_mha(nc):
    @contextlib.contextmanager
    def lab(label):
        if not LABELING:
            yield
            return
        before = {i.name for i in nc.all_instructions()}
        yield
        STAGE_LABELS.append(
            (label,
             [i.name for i in nc.all_instructions() if i.name not in before])
        )

    x_d = nc.dram_tensor("x", [S, D], F32, kind="ExternalInput")
    wqkv_d = nc.dram_tensor("W_qkv", [D, 3 * D], F32, kind="ExternalInput")
    wproj_d = nc.dram_tensor("W_proj", [D, D], F32, kind="ExternalInput")
    bproj_d = nc.dram_tensor("b_proj", [1, D], F32, kind="ExternalInput")
    out_d = nc.dram_tensor("out", [S, D], F32, kind="ExternalOutput")

    with tile.TileContext(nc) as tc:
        with (
            tc.tile_pool(name="persist", bufs=1) as pp,
            tc.tile_pool(name="psum", bufs=1, space="PSUM") as psum,
        ):
            def ptile():
                return psum.tile([128, 1024], F32, name="p1", tag="pbig",
                                 bufs=PBIG_BUFS)

            def ptiny():
                return psum.tile([128, 512], F32, name="pt1", tag="ptiny",
                                 bufs=PTINY_BUFS)

            def pavtile():
                if PAV_SHARED:
                    return ptiny()
                return psum.tile([128, 128], F32, name="pav1", tag="pav",
                                 bufs=2)

            # ---- constants (no DMAs yet: x gets the fabric first) ----
            ident_f32 = pp.tile([128, 128], F32, name="ident_f32", tag="ident_f32")
            make_identity(nc, ident_f32[:])
            ident_bf16 = pp.tile([128, 128], BF16, name="ident_bf16", tag="ident_bf16")
            nc.vector.tensor_copy(ident_bf16[:], ident_f32[:])
            ident_f32r = pp.tile([128, 128], F32R, name="ident_f32r", tag="ident_f32r")
            nc.vector.tensor_copy(ident_f32r[:], ident_f32[:])
            # bigmask: [0, S) cols zero; [S, S+128) = causal block with +1e10
            # on masked (k > q) entries.  (mask - s) * 8 -> masked = +8e10,
            # excluded from the min; exp(-(out) + bias) -> exp(-8e10) = 0.
            bigmask = pp.tile([128, 128], F32, name="bigmask", tag="bigmask")
            make_causal_mask(nc, bigmask[:], mask_val=-1e10)
            # transposed bf16 copy for PE mask-preload (lhsT so that
            # matmul(maskT, ident) writes bigmask into PSUM)
            bigmask_bf = pp.tile([128, 128], BF16, name="bigmask_bf",
                                 tag="bigmask_bf")
            nc.vector.tensor_copy(bigmask_bf[:], bigmask[:])
            maskT_bf = pp.tile([128, 128], BF16, name="maskT_bf", tag="maskT_bf")

            # ---- persistent activations ----
            qkT = [pp.tile([128, S], F32R, name=f"qkT{m}", tag=f"qkT{m}") for m in range(12)]
            v_sb = [pp.tile([128, H * HV], BF16, name=f"v{qi}", tag=f"v{qi}") for qi in range(NT)]
            attn = [pp.tile([128, D], BF16, name=f"attn{qi}", tag=f"attn{qi}") for qi in range(NT)]
            attnT = pp.tile([128, ND, S], BF16, name="attnT", tag="attnT")
            xT = pp.tile([128, ND, S], F32R, name="xT", tag="xT")
            wp = [pp.tile([128, D], BF16, name=f"wp{di}", tag=f"wp{di}") for di in range(ND)]
            wq = pp.tile([128, ND, 3 * D], F32R, name="wq", tag="wq")
            b_row = pp.tile([1, D], F32R, name="b_row", tag="b_row")
            ones_colf = pp.tile([1, 128], F32, name="ones_colf", tag="ones_colf")
            nc.vector.memset(ones_colf[:], 1.0)
            ones_col = pp.tile([1, 128], F32R, name="ones_col", tag="ones_col")
            nc.vector.tensor_copy(ones_col[:], ones_colf[:])
            b_bcast = pp.tile([128, D], F32, name="b_bcast", tag="b_bcast")

            # W_qkv as 18 column-slice DMAs [128, ND, 128] (512B runs):
            # slice m covers columns m*128..(m+1)*128 of every d-block.
            def wq_slice_dma(eng, m):
                eng.dma_start(
                    wq[:, :, m * 128:(m + 1) * 128],
                    wqkv_d.rearrange("(d p) c -> p d c", p=128)
                    [:, :, m * 128:(m + 1) * 128].bitcast(F32R),
                )

            for qi in range(NT):
                nc.gpsimd.memset(
                    v_sb[qi][:].rearrange("p (h v) -> p h v", v=HV)[:, :, HD:], 1.0
                )

            with (
                tc.tile_pool(name="xpool", bufs=3) as xp,
                tc.tile_pool(name="ph3", bufs=3) as p3,
                tc.tile_pool(name="ph3s", bufs=3) as p3s,
                tc.tile_pool(name="ypool", bufs=1) as yp,
            ):
                # ---- emitters ----
                xevac_eng = [nc.vector, nc.scalar]

                def x_chunk(qi):
                    x_t = xp.tile([128, D], F32R, name="x_t", tag="x_t")
                    nc.sync.dma_start(
                        x_t[:], x_d[qi * 128:(qi + 1) * 128, :].bitcast(F32R)
                    )
                    pxT = ptile()
                    for di in range(ND):
                        nc.tensor.transpose(
                            pxT[:, di * 128:(di + 1) * 128].bitcast(F32R),
                            x_t[:, di * 128:(di + 1) * 128],
                            ident_f32r[:],
                        )
                    eng = xevac_eng[qi % 2]
                    dst = xT[:, :, qi * 128:(qi + 1) * 128]
                    src = pxT[:, :D].rearrange("p (d q) -> p d q", q=128)
                    if eng is nc.scalar:
                        eng.copy(dst, src)
                    else:
                        eng.tensor_copy(dst, src)

                def v_proj(qi):
                    pv = ptile()
                    for c0, c1 in bank_chunks(D):
                        for di in range(ND):
                            nc.tensor.matmul(
                                pv[:, c0:c1],
                                xT[:, di, qi * 128:(qi + 1) * 128],
                                wq[:, di, 2 * D + c0:2 * D + c1],
                                start=(di == 0), stop=(di == ND - 1),
                            )
                    dst = v_sb[qi][:].rearrange("p (h v) -> p h v", v=HV)[:, :, :HD]
                    srcv = pv[:, :D].rearrange("p (h v) -> p h v", v=HD)
                    if qi % 2 == 0:
                        nc.vector.tensor_copy(dst, srcv)
                    else:
                        nc.scalar.copy(dst, srcv)

                def qk_half(m, half):
                    pqk = ptiny()
                    c0, c1 = (0, 512) if half == 0 else (512, 1024)
                    for di in range(ND):
                        nc.tensor.matmul(
                            pqk[:, 0:512],
                            wq[:, di, m * 128:(m + 1) * 128],
                            xT[:, di, c0:c1],
                            start=(di == 0), stop=(di == ND - 1),
                        )
                    if (m + half) % 2 == 0:
                        if m < 6:
                            nc.vector.tensor_scalar_mul(
                                qkT[m][:, c0:c1], pqk[:, 0:512], 8.0)
                        else:
                            nc.vector.tensor_copy(qkT[m][:, c0:c1], pqk[:, 0:512])
                    else:
                        if m < 6:
                            nc.scalar.mul(qkT[m][:, c0:c1], pqk[:, 0:512], 8.0)
                        else:
                            nc.scalar.copy(qkT[m][:, c0:c1], pqk[:, 0:512])

                def emit_b_bcast():
                    for c0, c1 in bank_chunks(D):
                        pb = ptiny()
                        nc.tensor.matmul(
                            pb[:, 0:c1 - c0], ones_col[:],
                            b_row[:, c0:c1],
                            start=True, stop=True,
                        )
                        nc.vector.tensor_copy(
                            b_bcast[:, c0:c1], pb[:, 0:c1 - c0]
                        )

                # ---- prologue ----
                ptm = ptiny().bitcast(BF16)
                nc.tensor.transpose(ptm[:, 0:128], bigmask_bf[:], ident_bf16[:])
                nc.vector.tensor_copy(maskT_bf[:], ptm[:, 0:128])
                wq_slice_dma(nc.scalar, 0)
                wq_slice_dma(nc.scalar, 6)
                with lab("xphase"):
                    for qi in range(4):
                        x_chunk(qi)
                with lab("qk0a"):
                    qk_half(0, 0)
                with lab("qk6a"):
                    qk_half(6, 0)
                with lab("xphase2"):
                    for qi in range(4, NT):
                        x_chunk(qi)
                # V slices + later Q/K slices + W_proj: held back with
                # clock waits so the greedy scheduler cannot let them steal
                # serial DMA bandwidth from x / m0 / m6.
                with tc.tile_wait_until(0.009):
                    for m in (12, 13, 14, 15, 16, 17):
                        wq_slice_dma(nc.scalar, m)
                with tc.tile_wait_until(0.016):
                    nc.scalar.dma_start(b_row[:], bproj_d[:].bitcast(F32R))
                for i, (ma, mb) in enumerate(
                        ((1, 7), (2, 8), (3, 9), (4, 10), (5, 11))):
                    with tc.tile_wait_until(0.017 + 0.003 * i):
                        wq_slice_dma(nc.scalar, ma)
                        wq_slice_dma(nc.scalar, mb)
                for di in range(ND):
                    with tc.tile_wait_until(0.032 + 0.001 * di):
                        # SWDGE cast fp32 -> bf16; needed only by proj
                        nc.gpsimd.dma_start(
                            wp[di][:],
                            wproj_d[di * 128:(di + 1) * 128, :],
                        )
                with lab("qk6b"):
                    qk_half(6, 1)
                with lab("qk0b"):
                    qk_half(0, 1)

                # ---- iteration order: heads 0..9 q-major, then last round
                # interleaves heads 10/11 so proj spreads over the round ----
                iters = [(h, qi) for h in range(10) for qi in range(NT)]
                for qi in range(NT):
                    iters.append((10, qi))
                    iters.append((11, qi))

                # step -> late weight DMAs / deferred setup
                dma_sched = {36: ("bbc", 0)}
                # step -> PE filler (V projections early, QK halves after)
                fill_sched = {}
                for qi in range(NT):
                    fill_sched[qi] = ("v", qi)
                _i = 0
                for m in (1, 7, 2, 8, 3, 9, 4, 10, 5, 11):
                    for half in (0, 1):
                        fill_sched[8 + 2 * _i] = ("qk", (m, half))
                        _i += 1

                # ---- attention software pipeline ----
                st = {}  # t -> dict of live tiles for iteration t

                def em_scores(t):
                    h, qi = iters[t]
                    qoff = (h % 2) * 64
                    Qt = qkT[h // 2]
                    Kt = qkT[6 + h // 2]
                    ks = (qi + 1) * 128
                    ps = ptiny() if (SMALL_PS and qi <= 3) else ptile()
                    lhs = Qt[qoff:qoff + 64, qi * 128:(qi + 1) * 128]
                    pe_mask = qi in PE_MASK_QIS
                    d0 = qi * 128
                    for c0, c1 in bank_chunks(ks):
                        if pe_mask and c1 > d0:
                            # chunk contains the diagonal block: PE writes the
                            # causal mask there, scores accumulate on top
                            if c0 < d0:
                                nc.tensor.matmul(
                                    ps[:, c0:d0], lhs,
                                    Kt[qoff:qoff + 64, c0:d0],
                                    start=True, stop=True,
                                )
                            nc.tensor.matmul(
                                ps[:, d0:c1], maskT_bf[:], ident_bf16[:],
                                start=True, stop=True, skip_group_check=True,
                            )
                            nc.tensor.matmul(
                                ps[:, d0:c1], lhs, Kt[qoff:qoff + 64, d0:c1],
                                start=False, stop=True, skip_group_check=True,
                            )
                        else:
                            nc.tensor.matmul(
                                ps[:, c0:c1], lhs, Kt[qoff:qoff + 64, c0:c1],
                                start=True, stop=True,
                            )
                    st[t] = {"ps": ps, "pe_mask": pe_mask}

                def em_maskmax(t):
                    h, qi = iters[t]
                    ks = (qi + 1) * 128
                    ps = st[t]["ps"]
                    if not st[t]["pe_mask"]:
                        nc.vector.tensor_tensor(
                            out=ps[:, qi * 128:ks],
                            in0=ps[:, qi * 128:ks],
                            in1=bigmask[:],
                            op=mybir.AluOpType.add,
                        )
                    neg8m = p3s.tile([128, 1], F32, name="neg8m", tag="neg8m", bufs=6)
                    nc.vector.reduce_max(
                        out=neg8m[:], in_=ps[:, :ks],
                        axis=mybir.AxisListType.X, negate=True,
                    )
                    st[t]["neg8m"] = neg8m

                def em_exp(t):
                    h, qi = iters[t]
                    ks = (qi + 1) * 128
                    w_t = p3s.tile([128, S], BF16, name="w_t", tag="w_t")
                    nc.scalar.activation(
                        w_t[:, :ks], st[t]["ps"][:, :ks],
                        mybir.ActivationFunctionType.Exp,
                        bias=st[t]["neg8m"][:], scale=1.0,
                    )
                    st[t]["w_t"] = w_t

                def em_transp(t):
                    h, qi = iters[t]
                    w_t = st[t]["w_t"]
                    pt8 = ptiny().bitcast(BF16).rearrange("p (b q) -> p b q", q=128)
                    for ki in range(qi + 1):
                        nc.tensor.transpose(
                            pt8[:, ki, :],
                            w_t[:, ki * 128:(ki + 1) * 128],
                            ident_bf16[:],
                        )
                    st[t]["pt8"] = pt8

                def em_wevac(t):
                    h, qi = iters[t]
                    wT = p3.tile([128, NT, 128], BF16, name="wT", tag="wT")
                    if WEVAC_PAT == "QI":
                        eng = nc.vector if qi >= 4 else nc.scalar
                    elif WEVAC_PAT == "QI2":
                        eng = nc.vector if qi % 2 else nc.scalar
                    else:
                        eng = nc.scalar if WEVAC_PAT[t % len(WEVAC_PAT)] == 'A' else nc.vector
                    if eng is nc.scalar:
                        eng.copy(wT[:, :qi + 1, :], st[t]["pt8"][:, :qi + 1, :])
                    else:
                        eng.tensor_copy(wT[:, :qi + 1, :], st[t]["pt8"][:, :qi + 1, :])
                    st[t]["wT"] = wT

                def em_av(t):
                    h, qi = iters[t]
                    po = pavtile()
                    for ki in range(qi + 1):
                        nc.tensor.matmul(
                            po[:, :HV],
                            st[t]["wT"][:, ki, :],
                            v_sb[ki][:, h * HV:(h + 1) * HV],
                            start=(ki == 0), stop=(ki == qi),
                        )
                    st[t]["po"] = po

                def em_scale(t):
                    h, qi = iters[t]
                    po = st[t]["po"]
                    recip = p3s.tile([128, 1], F32, name="recip", tag="recip",
                                     bufs=6)
                    nc.vector.reciprocal(recip[:], po[:, HD:HV])
                    dst = attn[qi][:, h * HD:(h + 1) * HD]
                    if SCALE_PAT[t % len(SCALE_PAT)] == 'A':
                        nc.scalar.mul(dst, po[:, :HD], recip[:])
                    else:
                        nc.vector.tensor_scalar_mul(dst, po[:, :HD], recip[:])
                    del st[t]

                def em_proj_a(qi):
                    pat = ptiny().bitcast(BF16).rearrange("p (b q) -> p b q", q=128)
                    for di in range(ND):
                        nc.tensor.transpose(
                            pat[:, di, :],
                            attn[qi][:, di * 128:(di + 1) * 128],
                            ident_bf16[:],
                        )
                    nc.scalar.copy(
                        attnT[:, :, qi * 128:(qi + 1) * 128], pat[:, :ND, :]
                    )

                def em_proj_b(qi):
                    y_t = yp.tile([128, D], F32, name="y_t", tag="y_t")
                    for c0, c1 in bank_chunks(D):
                        py = ptiny()
                        for di in range(ND):
                            nc.tensor.matmul(
                                py[:, 0:c1 - c0],
                                attnT[:, di, qi * 128:(qi + 1) * 128],
                                wp[di][:, c0:c1],
                                start=(di == 0), stop=(di == ND - 1),
                            )
                        nc.vector.tensor_tensor(
                            out=y_t[:, c0:c1], in0=py[:, 0:c1 - c0],
                            in1=b_bcast[:, c0:c1],
                            op=mybir.AluOpType.add,
                        )
                    nc.sync.dma_start(
                        out_d[qi * 128:(qi + 1) * 128, :], y_t[:]
                    )

                NIT = 12 * NT  # 96
                for s_step in range(NIT + SC_LEAD + AV_LAG + 4):
                    t_sc = s_step - SC_LEAD - AV_LAG - 1
                    if 0 <= t_sc < NIT:
                        with lab(f"scale{t_sc}"):
                            em_scale(t_sc)
                    t_pa = s_step - SC_LEAD - AV_LAG - 2
                    if 0 <= t_pa < NIT and iters[t_pa][0] == H - 1:
                        with lab(f"proja{iters[t_pa][1]}"):
                            em_proj_a(iters[t_pa][1])
                    t_pb = s_step - SC_LEAD - AV_LAG - 3
                    if 0 <= t_pb < NIT and iters[t_pb][0] == H - 1:
                        with lab(f"projb{iters[t_pb][1]}"):
                            em_proj_b(iters[t_pb][1])
                    if s_step < NIT:
                        with lab(f"scores{s_step}"):
                            em_scores(s_step)
                        with lab(f"maskmax{s_step}"):
                            em_maskmax(s_step)
                        with lab(f"exp{s_step}"):
                            em_exp(s_step)
                    t_tr = s_step - SC_LEAD
                    if 0 <= t_tr < NIT:
                        with lab(f"transp{t_tr}"):
                            em_transp(t_tr)
                        with lab(f"wevac{t_tr}"):
                            em_wevac(t_tr)
                    t_av = s_step - SC_LEAD - AV_LAG
                    if 0 <= t_av < NIT:
                        with lab(f"av{t_av}"):
                            em_av(t_av)
                    if dma_sched.get(s_step) is not None:
                        with lab("bbcast"):
                            emit_b_bcast()
                    fs = fill_sched.get(s_step)
                    if fs is not None:
                        kind, arg = fs
                        with lab(f"fill_{kind}{arg}"):
                            if kind == "v":
                                v_proj(arg)
                            else:
                                qk_half(*arg)

    nc.compile()
    return nc


_NC_CACHE = None


def _get_nc():
    global _NC_CACHE
    if _NC_CACHE is None:
        nc = bacc.Bacc(
            "TRN2",
            target_bir_lowering=False,
            debug=False,
            num_devices=N_CORES,
        )
        build_mha(nc)
        _NC_CACHE = nc
    return _NC_CACHE


def kernel(x, W_qkv, W_proj, b_proj):
    nc = _get_nc()
    x = np.ascontiguousarray(np.asarray(x, dtype=np.float32))
    W_qkv = np.ascontiguousarray(np.asarray(W_qkv, dtype=np.float32))
    W_proj = np.ascontiguousarray(np.asarray(W_proj, dtype=np.float32))
    b_proj = np.ascontiguousarray(
        np.asarray(b_proj, dtype=np.float32).reshape(1, D)
    )
    in_maps = [
        {"x": x[b], "W_qkv": W_qkv, "W_proj": W_proj, "b_proj": b_proj}
        for b in range(N_CORES)
    ]
    res = run_bass_kernel_spmd(nc, in_maps, core_ids=list(range(N_CORES)))
    return np.stack([res.results[b]["out"] for b in range(N_CORES)], axis=0)
